# revision 2
# baseline (speedup 1.0000x reference)
"""Trainium2 Bass kernel for nn_CCA_Block (cross-channel attention block).

Reference computation (per batch element, B=8 sharded one-per-core):
    q = relu(x1 @ Wq); k = relu(x1 @ Wk); v = relu(x2 @ Wv)      # 1x1 convs
    scores[c,h,g] = scale * sum_w q[h,w,c] * k[g,w,c]
    attn = softmax(scores, axis=g)
    o[h,w,c] = sum_g attn[c,h,g] * v[g,w,c]
    g = sigmoid(o @ Ws + bs)
    g = gamma * (g - mu) / sqrt(var + eps) + beta
    out = x1 + x2 * g

Sharding: data-parallel over batch across the 8 NeuronCores (batch b -> core b).

Per-core dataflow (all matmuls bf16 with fp32 PSUM accumulate):
  V:  x2 w-major cast-DMA -> DMA-transpose -> x2T -> PE conv -> relu -> v_sb
  QK: x1 h-major cast-DMA -> DMA-transpose -> x1T -> PE conv -> relu -> q_sb,k_sb
  A:  per channel c: scoresT = kT_c^T' qT_c (PE) -> exp (ACT, scale folded)
      -> o_unnorm = E^T V_c and Z = E^T 1 (PE, one PSUM group)
      -> 1/Z (DVE) -> o = o_unnorm * 1/Z + delta  (delta solves Ws^T d = bs)
  G:  o -> oT (PE transpose) -> z = oT^T Ws (PE) -> sigmoid (ACT)
      -> BN affine (host-folded a,b) -> out = x1 + x2*g (x1 re-read fp32)
"""

import numpy as np
import ml_dtypes

B, H, W, C = 8, 128, 128, 128
N_CORES = 8
BN_EPS = 1e-3

_BUILD_CACHE: dict = {}


def _build_program(scale_val: float, delta: tuple, bias_via_dve: bool, b_zero: bool):
    """Emit + compile the per-core Bass program. All cores run the identical
    program on their own batch slice."""
    import concourse.bacc as bacc
    import concourse.mybir as mybir
    import concourse.tile as tile

    fp32 = mybir.dt.float32
    bf16 = mybir.dt.bfloat16
    AF = mybir.ActivationFunctionType
    OP = mybir.AluOpType

    nc = bacc.Bacc("TRN2", target_bir_lowering=False, debug=False,
                   enable_asserts=False)

    x1_d = nc.dram_tensor("x1", [H, W, C], fp32, kind="ExternalInput")
    x2_d = nc.dram_tensor("x2", [H, W, C], fp32, kind="ExternalInput")
    wq_d = nc.dram_tensor("wq", [C, C], bf16, kind="ExternalInput")
    wk_d = nc.dram_tensor("wk", [C, C], bf16, kind="ExternalInput")
    wv_d = nc.dram_tensor("wv", [C, C], bf16, kind="ExternalInput")
    ws_d = nc.dram_tensor("ws", [C, C], bf16, kind="ExternalInput")
    ones_d = nc.dram_tensor("ones_col", [C, 1], bf16, kind="ExternalInput")
    ident_d = nc.dram_tensor("ident", [C, C], bf16, kind="ExternalInput")
    arep_d = nc.dram_tensor("a_rep", [C, 4 * C], bf16, kind="ExternalInput")
    brep_d = nc.dram_tensor("b_rep", [C, 4 * C], bf16, kind="ExternalInput")
    bsrep_d = nc.dram_tensor("bs_rep", [C, 4 * C], fp32, kind="ExternalInput")
    out_d = nc.dram_tensor("out", [H, W, C], fp32, kind="ExternalOutput")

    x1_ap, x2_ap, out_ap = x1_d.ap(), x2_d.ap(), out_d.ap()

    with tile.TileContext(nc) as tc:
        with (
            # persistent single-buffer pools
            tc.tile_pool(name="wts", bufs=1) as p_wts,
            tc.tile_pool(name="qkv", bufs=1) as p_qkv,
            tc.tile_pool(name="obuf", bufs=1) as p_o,
            # streaming pools
            tc.tile_pool(name="xcast", bufs=3) as p_xcast,
            tc.tile_pool(name="xT", bufs=6) as p_xT,
            tc.tile_pool(name="eexp", bufs=3) as p_e,
            tc.tile_pool(name="rz", bufs=3) as p_rz,
            tc.tile_pool(name="oT", bufs=3) as p_oT,
            tc.tile_pool(name="gres", bufs=3) as p_g,
            tc.tile_pool(name="x1w", bufs=3) as p_x1w,
            tc.tile_pool(name="outt", bufs=3) as p_out,
            # psum: shared full-bank tag (4 banks) + small tags (2+2 banks)
            tc.tile_pool(name="psA", bufs=4, space="PSUM") as ps_a,
            tc.tile_pool(name="psB", bufs=2, space="PSUM") as ps_b,
        ):
            # ---- constants ----
            wq = p_wts.tile([C, C], bf16, tag="wq")
            wk = p_wts.tile([C, C], bf16, tag="wk")
            wv = p_wts.tile([C, C], bf16, tag="wv")
            ws = p_wts.tile([C, C], bf16, tag="ws")
            ones = p_wts.tile([C, 1], bf16, tag="ones")
            ident = p_wts.tile([C, C], bf16, tag="ident")
            arep = p_wts.tile([C, 4 * C], bf16, tag="arep")
            nc.sync.dma_start(wq[:], wq_d.ap())
            nc.sync.dma_start(wk[:], wk_d.ap())
            nc.sync.dma_start(wv[:], wv_d.ap())
            nc.sync.dma_start(ws[:], ws_d.ap())
            nc.sync.dma_start(ones[:], ones_d.ap())
            nc.sync.dma_start(ident[:], ident_d.ap())
            nc.sync.dma_start(arep[:], arep_d.ap())
            if not b_zero:
                brep = p_wts.tile([C, 4 * C], bf16, tag="brep")
                nc.sync.dma_start(brep[:], brep_d.ap())
            if bias_via_dve:
                bsrep = p_wts.tile([C, 4 * C], fp32, tag="bsrep")
                nc.sync.dma_start(bsrep[:], bsrep_d.ap())

            # persistent big buffers (bf16): free-axis layouts noted
            q_sb = p_qkv.tile([W, H * C], bf16, tag="q")    # [w, h*128+c]
            k_sb = p_qkv.tile([W, H * C], bf16, tag="k")    # [w, h*128+c]
            v_sb = p_qkv.tile([H, W * C], bf16, tag="v")    # [g, w*128+c]
            o_sb = p_o.tile([H, C * W], bf16, tag="o")      # [h, c*128+w]

            # ============ Phase V: x2 -> x2T -> v ============
            for w0 in range(0, W, 4):
                xc2 = p_xcast.tile([H, 4 * C], bf16, tag="xc2")
                nc.gpsimd.dma_start(xc2[:], x2_ap[:, w0 : w0 + 4, :])
                psv = ps_a.tile([H, 512], fp32, tag="ps")
                for j in range(4):
                    x2T = p_xT.tile([C, H], bf16, tag="xT")
                    nc.sync.dma_start(
                        x2T[:], xc2[:, j * C : (j + 1) * C], transpose=True
                    )
                    nc.tensor.matmul(
                        psv[:, j * C : (j + 1) * C], x2T[:], wv[:],
                        start=(j == 0), stop=(j == 3),
                    )
                nc.vector.tensor_scalar(
                    v_sb[:, w0 * C : (w0 + 4) * C], psv[:], 0.0, None, OP.max
                )

            # ============ Phase QK: x1 -> x1T -> q,k ============
            for h0 in range(0, H, 4):
                xc = p_xcast.tile([W, 4 * C], bf16, tag="xc")
                src = x1_ap[h0 : h0 + 4].rearrange("hh w c -> w hh c")
                nc.gpsimd.dma_start(xc[:], src)
                psq = ps_a.tile([W, 512], fp32, tag="ps")
                psk = ps_a.tile([W, 512], fp32, tag="ps")
                for j in range(4):
                    x1T = p_xT.tile([C, W], bf16, tag="xT")
                    nc.sync.dma_start(
                        x1T[:], xc[:, j * C : (j + 1) * C], transpose=True
                    )
                    nc.tensor.matmul(
                        psq[:, j * C : (j + 1) * C], x1T[:], wq[:],
                        start=(j == 0), stop=(j == 3),
                    )
                    nc.tensor.matmul(
                        psk[:, j * C : (j + 1) * C], x1T[:], wk[:],
                        start=(j == 0), stop=(j == 3),
                    )
                blk = h0 * C
                nc.scalar.activation(q_sb[:, blk : blk + 512], psq[:], AF.Relu)
                nc.vector.tensor_scalar(
                    k_sb[:, blk : blk + 512], psk[:], 0.0, None, OP.max
                )

            # ============ Phase A: attention over channels ============
            q3 = q_sb[:].rearrange("w (h c) -> w h c", c=C)
            k3 = k_sb[:].rearrange("w (h c) -> w h c", c=C)
            v3 = v_sb[:].rearrange("g (w c) -> g w c", c=C)
            for c0 in range(0, C, 4):
                pss = ps_a.tile([H, 512], fp32, tag="ps")
                for j in range(4):
                    c = c0 + j
                    nc.tensor.matmul(
                        pss[:, j * H : (j + 1) * H],
                        k3[:, :, c], q3[:, :, c],
                        start=(j == 0), stop=(j == 3),
                    )
                e4 = p_e.tile([H, 512], bf16, tag="e4")
                nc.scalar.activation(e4[:], pss[:], AF.Exp, scale=scale_val)
                pso = ps_a.tile([H, 512], fp32, tag="ps")
                psz = ps_b.tile([H, 4], fp32, tag="psz")
                for j in range(4):
                    c = c0 + j
                    eT = e4[:, j * H : (j + 1) * H]
                    nc.tensor.matmul(
                        pso[:, j * C : (j + 1) * C], eT, v3[:, :, c],
                        start=(j == 0), stop=(j == 3),
                    )
                    nc.tensor.matmul(
                        psz[:, j : j + 1], eT, ones[:],
                        start=(j == 0), stop=(j == 3),
                    )
                rz = p_rz.tile([H, 4], fp32, tag="rz")
                nc.vector.reciprocal(rz[:], psz[:])
                for j in range(4):
                    c = c0 + j
                    dst = o_sb[:, c * W : (c + 1) * W]
                    src = pso[:, j * C : (j + 1) * C]
                    if (c0 // 4) % 2 == 0:
                        nc.scalar.activation(
                            dst, src, AF.Copy,
                            bias=float(delta[c]), scale=rz[:, j : j + 1],
                        )
                    else:
                        nc.vector.tensor_scalar(
                            dst, src, rz[:, j : j + 1], float(delta[c]),
                            OP.mult, OP.add,
                        )

            # ============ Phase G: o -> oT -> conv -> sigmoid/BN/residual ====
            o3 = o_sb[:].rearrange("h (c w) -> h c w", w=W)
            for w0 in range(0, W, 4):
                pst = ps_b.tile([C, 512], bf16, tag="pst")
                for j in range(4):
                    w = w0 + j
                    nc.tensor.matmul(
                        pst[:, j * H : (j + 1) * H], o3[:, :, w], ident[:],
                        is_transpose=True, start=(j == 0), stop=(j == 3),
                    )
                oT = p_oT.tile([C, 512], bf16, tag="oT")
                nc.vector.tensor_copy(oT[:], pst[:])
                psg = ps_a.tile([H, 512], fp32, tag="ps")
                for j in range(4):
                    nc.tensor.matmul(
                        psg[:, j * C : (j + 1) * C],
                        oT[:, j * H : (j + 1) * H], ws[:],
                        start=(j == 0), stop=(j == 3),
                    )
                if bias_via_dve:
                    nc.vector.tensor_tensor(psg[:], psg[:], bsrep[:], OP.add)
                g4 = p_g.tile([H, 512], bf16, tag="g4")
                nc.scalar.activation(g4[:], psg[:], AF.Sigmoid)
                gbn = p_g.tile([H, 512], bf16, tag="gbn")
                nc.gpsimd.tensor_tensor(gbn[:], g4[:], arep[:], OP.mult)
                if not b_zero:
                    nc.gpsimd.tensor_tensor(gbn[:], gbn[:], brep[:], OP.add)
                xc2g = p_xcast.tile([H, 4 * C], bf16, tag="xc2")
                nc.gpsimd.dma_start(xc2g[:], x2_ap[:, w0 : w0 + 4, :])
                x1w = p_x1w.tile([H, 512], fp32, tag="x1w")
                nc.sync.dma_start(x1w[:], x1_ap[:, w0 : w0 + 4, :])
                t4 = p_g.tile([H, 512], fp32, tag="t4")
                nc.vector.tensor_tensor(t4[:], xc2g[:], gbn[:], OP.mult)
                o4 = p_out.tile([H, 512], fp32, tag="o4")
                nc.vector.tensor_tensor(o4[:], t4[:], x1w[:], OP.add)
                nc.sync.dma_start(out_ap[:, w0 : w0 + 4, :], o4[:])

    nc.compile()
    return nc


def _prepare(inputs):
    """Host-side prep: derived small tensors + baked scalars."""
    x1 = np.ascontiguousarray(np.asarray(inputs["x1"], dtype=np.float32))
    x2 = np.ascontiguousarray(np.asarray(inputs["x2"], dtype=np.float32))
    Wq = np.asarray(inputs["Wq"], dtype=np.float32)
    Wk = np.asarray(inputs["Wk"], dtype=np.float32)
    Wv = np.asarray(inputs["Wv"], dtype=np.float32)
    Ws = np.asarray(inputs["Ws"], dtype=np.float32)
    bs = np.asarray(inputs["bs"], dtype=np.float32)
    scale = float(np.asarray(inputs["scale"]).reshape(-1)[0])
    gamma = np.asarray(inputs["gamma"], dtype=np.float32)
    beta = np.asarray(inputs["beta"], dtype=np.float32)
    mu = np.asarray(inputs["mu"], dtype=np.float32)
    var = np.asarray(inputs["var"], dtype=np.float32)

    a = gamma / np.sqrt(var + BN_EPS)
    b = beta - mu * a
    b_zero = bool(np.all(b == 0.0))

    # fold the sigmoid bias bs into o:  o' = o + delta with Ws^T delta = bs
    bias_via_dve = False
    delta = np.zeros(C, dtype=np.float64)
    if np.any(bs != 0.0):
        try:
            delta = np.linalg.solve(Ws.astype(np.float64).T, bs.astype(np.float64))
            resid = np.abs(Ws.T @ delta.astype(np.float32) - bs).max()
            if not np.isfinite(delta).all() or resid > 1e-5 * (1 + np.abs(bs).max()):
                raise np.linalg.LinAlgError("bad solve")
        except np.linalg.LinAlgError:
            delta = np.zeros(C, dtype=np.float64)
            bias_via_dve = True

    bf = ml_dtypes.bfloat16
    consts = {
        "wq": Wq.astype(bf),
        "wk": Wk.astype(bf),
        "wv": Wv.astype(bf),
        "ws": Ws.astype(bf),
        "ones_col": np.ones((C, 1), dtype=bf),
        "ident": np.eye(C, dtype=bf),
        "a_rep": np.tile(a, (C, 4)).astype(bf),
        "b_rep": np.tile(b, (C, 4)).astype(bf),
        "bs_rep": np.tile(bs, (C, 4)).astype(np.float32),
    }
    key = (scale, tuple(np.round(delta, 12)), bias_via_dve, b_zero)
    return x1, x2, consts, key, scale, delta, bias_via_dve, b_zero


def _get_nc(key, scale, delta, bias_via_dve, b_zero):
    if key not in _BUILD_CACHE:
        _BUILD_CACHE[key] = _build_program(scale, delta, bias_via_dve, b_zero)
    return _BUILD_CACHE[key]


def run(inputs, trace: bool = False):
    from concourse.bass_utils import run_bass_kernel_spmd

    x1, x2, consts, key, scale, delta, bias_via_dve, b_zero = _prepare(inputs)
    nc = _get_nc(key, scale, delta, bias_via_dve, b_zero)

    in_maps = []
    for core in range(N_CORES):
        m = dict(consts)
        m["x1"] = x1[core]
        m["x2"] = x2[core]
        in_maps.append(m)

    res = run_bass_kernel_spmd(
        nc, in_maps, core_ids=list(range(N_CORES)), trace=trace
    )
    out = np.stack([res.results[i]["out"] for i in range(N_CORES)], axis=0)
    return out.astype(np.float32), res


def kernel(**inputs) -> np.ndarray:
    out, _ = run(inputs, trace=False)
    return out


# revision 3
# speedup vs baseline: 2.3289x; 2.3289x over previous
"""Trainium2 Bass kernel for nn_CCA_Block (cross-channel attention block).

Reference computation (per batch element, B=8 sharded one-per-core):
    q = relu(x1 @ Wq); k = relu(x1 @ Wk); v = relu(x2 @ Wv)      # 1x1 convs
    scores[c,h,g] = scale * sum_w q[h,w,c] * k[g,w,c]
    attn = softmax(scores, axis=g)
    o[h,w,c] = sum_g attn[c,h,g] * v[g,w,c]
    g = sigmoid(o @ Ws + bs)
    g = gamma * (g - mu) / sqrt(var + eps) + beta
    out = x1 + x2 * g

Sharding: data-parallel over batch across the 8 NeuronCores (batch b -> core b).

Per-core dataflow (matmuls in bf16 with fp32 PSUM accumulate; transposes are
PE transpose-mode matmuls batched 4-wide into bf16 PSUM):
  V:  x2 w-major cast-DMA -> PE transpose -> x2T -> PE conv -> relu -> v_sb
  QK: x1 h-major cast-DMA -> PE transpose -> x1T -> PE conv -> relu -> q_sb,k_sb
  A:  per channel c: scoresT = kT_c' qT_c (PE) -> exp (ACT, scale folded)
      -> o_unnorm = E' V_c and Z = E' 1 (PE, adjacent PSUM groups)
      -> 1/Z (DVE) -> o = o_unnorm * (1/Z bcast) + delta  (Ws^T delta = bs)
  G:  o -> oT (PE transpose) -> z = oT' Ws (PE) -> sigmoid (ACT)
      -> BN affine (host-folded a,b) -> t = x2*g -> t += x1 (DMA accumulate)
"""

import numpy as np
import ml_dtypes

B, H, W, C = 8, 128, 128, 128
N_CORES = 8
BN_EPS = 1e-3

_BUILD_CACHE: dict = {}


def _build_program(scale_val: float, delta: tuple, bias_via_dve: bool, b_zero: bool):
    """Emit + compile the per-core Bass program. All cores run the identical
    program on their own batch slice."""
    import concourse.bacc as bacc
    import concourse.mybir as mybir
    import concourse.tile as tile

    fp32 = mybir.dt.float32
    bf16 = mybir.dt.bfloat16
    AF = mybir.ActivationFunctionType
    OP = mybir.AluOpType
    delta_zero = all(d == 0.0 for d in delta)

    nc = bacc.Bacc("TRN2", target_bir_lowering=False, debug=False,
                   enable_asserts=False)

    x1_d = nc.dram_tensor("x1", [H, W, C], fp32, kind="ExternalInput")
    x2_d = nc.dram_tensor("x2", [H, W, C], fp32, kind="ExternalInput")
    wq_d = nc.dram_tensor("wq", [C, C], bf16, kind="ExternalInput")
    wk_d = nc.dram_tensor("wk", [C, C], bf16, kind="ExternalInput")
    wv_d = nc.dram_tensor("wv", [C, C], bf16, kind="ExternalInput")
    ws_d = nc.dram_tensor("ws", [C, C], bf16, kind="ExternalInput")
    ones_d = nc.dram_tensor("ones_col", [C, 1], bf16, kind="ExternalInput")
    ident_d = nc.dram_tensor("ident", [C, C], bf16, kind="ExternalInput")
    arep_d = nc.dram_tensor("a_rep", [C, 4 * C], bf16, kind="ExternalInput")
    brep_d = nc.dram_tensor("b_rep", [C, 4 * C], bf16, kind="ExternalInput")
    bsrep_d = nc.dram_tensor("bs_rep", [C, 4 * C], fp32, kind="ExternalInput")
    out_d = nc.dram_tensor("out", [H, W, C], fp32, kind="ExternalOutput")

    x1_ap, x2_ap, out_ap = x1_d.ap(), x2_d.ap(), out_d.ap()

    with tile.TileContext(nc) as tc:
        with (
            # persistent single-buffer pools
            tc.tile_pool(name="wts", bufs=1) as p_wts,
            tc.tile_pool(name="qkv", bufs=1) as p_qkv,
            tc.tile_pool(name="obuf", bufs=1) as p_o,
            # streaming pools
            tc.tile_pool(name="xcast", bufs=3) as p_xcast,
            tc.tile_pool(name="xT", bufs=3) as p_xT,
            tc.tile_pool(name="eexp", bufs=3) as p_e,
            tc.tile_pool(name="rz", bufs=3) as p_rz,
            tc.tile_pool(name="oT", bufs=3) as p_oT,
            tc.tile_pool(name="gres", bufs=3) as p_g,
            tc.tile_pool(name="x2f", bufs=3) as p_x2f,
            tc.tile_pool(name="outt", bufs=3) as p_out,
            # psum: shared full-bank fp32 tag (4) + bf16 transpose tag (2)
            # + tiny Z tag (2) = 8 banks
            tc.tile_pool(name="psA", bufs=4, space="PSUM") as ps_a,
            tc.tile_pool(name="psT", bufs=2, space="PSUM") as ps_t,
            tc.tile_pool(name="psZ", bufs=2, space="PSUM") as ps_z,
        ):
            # ---- constants ----
            wq = p_wts.tile([C, C], bf16, tag="wq")
            wk = p_wts.tile([C, C], bf16, tag="wk")
            wv = p_wts.tile([C, C], bf16, tag="wv")
            ws = p_wts.tile([C, C], bf16, tag="ws")
            ones = p_wts.tile([C, 1], bf16, tag="ones")
            ident = p_wts.tile([C, C], bf16, tag="ident")
            arep = p_wts.tile([C, 4 * C], bf16, tag="arep")
            nc.sync.dma_start(wq[:], wq_d.ap())
            nc.sync.dma_start(wk[:], wk_d.ap())
            nc.sync.dma_start(wv[:], wv_d.ap())
            nc.sync.dma_start(ws[:], ws_d.ap())
            nc.sync.dma_start(ones[:], ones_d.ap())
            nc.sync.dma_start(ident[:], ident_d.ap())
            nc.sync.dma_start(arep[:], arep_d.ap())
            if not b_zero:
                brep = p_wts.tile([C, 4 * C], bf16, tag="brep")
                nc.sync.dma_start(brep[:], brep_d.ap())
            if bias_via_dve:
                bsrep = p_wts.tile([C, 4 * C], fp32, tag="bsrep")
                nc.sync.dma_start(bsrep[:], bsrep_d.ap())

            # persistent big buffers (bf16): free-axis layouts noted
            q_sb = p_qkv.tile([W, H * C], bf16, tag="q")    # [w, h*128+c]
            k_sb = p_qkv.tile([W, H * C], bf16, tag="k")    # [w, h*128+c]
            v_sb = p_qkv.tile([H, W * C], bf16, tag="v")    # [g, w*128+c]
            o_sb = p_o.tile([H, C * W], bf16, tag="o")      # [h, c*128+w]

            def transpose4(src_fn, evac_engine):
                """4 PE tile-transposes into one bf16 PSUM bank + wide evac.
                src_fn(j) -> [128,128] bf16 SBUF AP. Returns SBUF tile
                [128, 512] holding the 4 transposed tiles."""
                pst = ps_t.tile([C, 512], bf16, tag="pst")
                for j in range(4):
                    nc.tensor.matmul(
                        pst[:, j * C : (j + 1) * C], src_fn(j), ident[:],
                        is_transpose=True, start=(j == 0), stop=(j == 3),
                    )
                xt = p_xT.tile([C, 512], bf16, tag="xT")
                if evac_engine == "act":
                    nc.scalar.activation(xt[:], pst[:], AF.Copy)
                else:
                    nc.vector.tensor_copy(xt[:], pst[:])
                return xt

            # ============ Phase V: x2 -> x2T -> v ============
            for w0 in range(0, W, 4):
                xc2 = p_xcast.tile([H, 4 * C], bf16, tag="xc2")
                nc.gpsimd.dma_start(xc2[:], x2_ap[:, w0 : w0 + 4, :])
                x2T = transpose4(
                    lambda j: xc2[:, j * C : (j + 1) * C], "dve"
                )
                psv = ps_a.tile([H, 512], fp32, tag="ps")
                for j in range(4):
                    nc.tensor.matmul(
                        psv[:, j * C : (j + 1) * C],
                        x2T[:, j * C : (j + 1) * C], wv[:],
                        start=(j == 0), stop=(j == 3),
                    )
                nc.scalar.activation(
                    v_sb[:, w0 * C : (w0 + 4) * C], psv[:], AF.Relu
                )

            # ============ Phase QK: x1 -> x1T -> q,k ============
            for h0 in range(0, H, 4):
                xc = p_xcast.tile([W, 4 * C], bf16, tag="xc")
                src = x1_ap[h0 : h0 + 4].rearrange("hh w c -> w hh c")
                nc.gpsimd.dma_start(xc[:], src)
                x1T = transpose4(
                    lambda j: xc[:, j * C : (j + 1) * C], "act"
                )
                psq = ps_a.tile([W, 512], fp32, tag="ps")
                psk = ps_a.tile([W, 512], fp32, tag="ps")
                for j in range(4):
                    xTj = x1T[:, j * C : (j + 1) * C]
                    nc.tensor.matmul(
                        psq[:, j * C : (j + 1) * C], xTj, wq[:],
                        start=(j == 0), stop=(j == 3),
                    )
                    nc.tensor.matmul(
                        psk[:, j * C : (j + 1) * C], xTj, wk[:],
                        start=(j == 0), stop=(j == 3),
                    )
                blk = h0 * C
                nc.scalar.activation(q_sb[:, blk : blk + 512], psq[:], AF.Relu)
                nc.vector.tensor_scalar(
                    k_sb[:, blk : blk + 512], psk[:], 0.0, None, OP.max
                )

            # ============ Phase A: attention over channels ============
            q3 = q_sb[:].rearrange("w (h c) -> w h c", c=C)
            k3 = k_sb[:].rearrange("w (h c) -> w h c", c=C)
            v3 = v_sb[:].rearrange("g (w c) -> g w c", c=C)
            for c0 in range(0, C, 4):
                pss = ps_a.tile([H, 512], fp32, tag="ps")
                for j in range(4):
                    c = c0 + j
                    nc.tensor.matmul(
                        pss[:, j * H : (j + 1) * H],
                        k3[:, :, c], q3[:, :, c],
                        start=(j == 0), stop=(j == 3),
                    )
                e4 = p_e.tile([H, 512], bf16, tag="e4")
                nc.scalar.activation(e4[:], pss[:], AF.Exp, scale=scale_val)
                pso = ps_a.tile([H, 512], fp32, tag="ps")
                psz = ps_z.tile([H, 4], fp32, tag="psz")
                for j in range(4):
                    c = c0 + j
                    eT = e4[:, j * H : (j + 1) * H]
                    nc.tensor.matmul(
                        pso[:, j * C : (j + 1) * C], eT, v3[:, :, c],
                        start=(j == 0), stop=(j == 3),
                    )
                    nc.tensor.matmul(
                        psz[:, j : j + 1], eT, ones[:],
                        start=(j == 0), stop=(j == 3),
                    )
                rz = p_rz.tile([H, 4], fp32, tag="rz")
                nc.vector.reciprocal(rz[:], psz[:])
                if delta_zero:
                    # wide normalize: o = o_unnorm * (1/Z) with 1/Z
                    # broadcast along w via a stride-0 AP
                    rzb = rz[:].unsqueeze(2).broadcast_to([H, 4, C])
                    nc.vector.tensor_tensor(
                        o_sb[:, c0 * W : (c0 + 4) * W], pso[:], rzb, OP.mult
                    )
                else:
                    for j in range(4):
                        c = c0 + j
                        dst = o_sb[:, c * W : (c + 1) * W]
                        src = pso[:, j * C : (j + 1) * C]
                        if (c0 // 4) % 2 == 0:
                            nc.scalar.activation(
                                dst, src, AF.Copy,
                                bias=float(delta[c]), scale=rz[:, j : j + 1],
                            )
                        else:
                            nc.vector.tensor_scalar(
                                dst, src, rz[:, j : j + 1], float(delta[c]),
                                OP.mult, OP.add,
                            )

            # ============ Phase G: o -> oT -> conv -> sigmoid/BN/residual ====
            o3 = o_sb[:].rearrange("h (c w) -> h c w", w=W)
            for w0 in range(0, W, 4):
                oT = transpose4(lambda j: o3[:, :, w0 + j], "dve")
                psg = ps_a.tile([H, 512], fp32, tag="ps")
                for j in range(4):
                    nc.tensor.matmul(
                        psg[:, j * C : (j + 1) * C],
                        oT[:, j * H : (j + 1) * H], ws[:],
                        start=(j == 0), stop=(j == 3),
                    )
                if bias_via_dve:
                    nc.vector.tensor_tensor(psg[:], psg[:], bsrep[:], OP.add)
                g4 = p_g.tile([H, 512], bf16, tag="g4")
                nc.scalar.activation(g4[:], psg[:], AF.Sigmoid)
                gbn = p_g.tile([H, 512], bf16, tag="gbn")
                nc.vector.tensor_tensor(gbn[:], g4[:], arep[:], OP.mult)
                if not b_zero:
                    nc.vector.tensor_tensor(gbn[:], gbn[:], brep[:], OP.add)
                x2f = p_x2f.tile([H, 512], fp32, tag="x2f")
                nc.sync.dma_start(x2f[:], x2_ap[:, w0 : w0 + 4, :])
                t4 = p_out.tile([H, 512], fp32, tag="t4")
                if w0 % 8 == 0:
                    nc.vector.tensor_tensor(t4[:], x2f[:], gbn[:], OP.mult)
                else:
                    nc.gpsimd.tensor_tensor(t4[:], x2f[:], gbn[:], OP.mult)
                # residual add: t4 += x1 via SWDGE accumulate DMA
                nc.gpsimd.dma_start(
                    t4[:], x1_ap[:, w0 : w0 + 4, :], accum_op=OP.add
                )
                nc.sync.dma_start(out_ap[:, w0 : w0 + 4, :], t4[:])

    nc.compile()
    return nc


def _prepare(inputs):
    """Host-side prep: derived small tensors + baked scalars."""
    x1 = np.ascontiguousarray(np.asarray(inputs["x1"], dtype=np.float32))
    x2 = np.ascontiguousarray(np.asarray(inputs["x2"], dtype=np.float32))
    Wq = np.asarray(inputs["Wq"], dtype=np.float32)
    Wk = np.asarray(inputs["Wk"], dtype=np.float32)
    Wv = np.asarray(inputs["Wv"], dtype=np.float32)
    Ws = np.asarray(inputs["Ws"], dtype=np.float32)
    bs = np.asarray(inputs["bs"], dtype=np.float32)
    scale = float(np.asarray(inputs["scale"]).reshape(-1)[0])
    gamma = np.asarray(inputs["gamma"], dtype=np.float32)
    beta = np.asarray(inputs["beta"], dtype=np.float32)
    mu = np.asarray(inputs["mu"], dtype=np.float32)
    var = np.asarray(inputs["var"], dtype=np.float32)

    a = gamma / np.sqrt(var + BN_EPS)
    b = beta - mu * a
    b_zero = bool(np.all(b == 0.0))

    # fold the sigmoid bias bs into o:  o' = o + delta with Ws^T delta = bs
    bias_via_dve = False
    delta = np.zeros(C, dtype=np.float64)
    if np.any(bs != 0.0):
        try:
            delta = np.linalg.solve(Ws.astype(np.float64).T, bs.astype(np.float64))
            resid = np.abs(Ws.T @ delta.astype(np.float32) - bs).max()
            if not np.isfinite(delta).all() or resid > 1e-5 * (1 + np.abs(bs).max()):
                raise np.linalg.LinAlgError("bad solve")
        except np.linalg.LinAlgError:
            delta = np.zeros(C, dtype=np.float64)
            bias_via_dve = True

    bf = ml_dtypes.bfloat16
    consts = {
        "wq": Wq.astype(bf),
        "wk": Wk.astype(bf),
        "wv": Wv.astype(bf),
        "ws": Ws.astype(bf),
        "ones_col": np.ones((C, 1), dtype=bf),
        "ident": np.eye(C, dtype=bf),
        "a_rep": np.tile(a, (C, 4)).astype(bf),
        "b_rep": np.tile(b, (C, 4)).astype(bf),
        "bs_rep": np.tile(bs, (C, 4)).astype(np.float32),
    }
    key = (scale, tuple(np.round(delta, 12)), bias_via_dve, b_zero)
    return x1, x2, consts, key, scale, delta, bias_via_dve, b_zero


def _get_nc(key, scale, delta, bias_via_dve, b_zero):
    if key not in _BUILD_CACHE:
        _BUILD_CACHE[key] = _build_program(scale, delta, bias_via_dve, b_zero)
    return _BUILD_CACHE[key]


def run(inputs, trace: bool = False):
    from concourse.bass_utils import run_bass_kernel_spmd

    x1, x2, consts, key, scale, delta, bias_via_dve, b_zero = _prepare(inputs)
    nc = _get_nc(key, scale, delta, bias_via_dve, b_zero)

    in_maps = []
    for core in range(N_CORES):
        m = dict(consts)
        m["x1"] = x1[core]
        m["x2"] = x2[core]
        in_maps.append(m)

    res = run_bass_kernel_spmd(
        nc, in_maps, core_ids=list(range(N_CORES)), trace=trace
    )
    out = np.stack([res.results[i]["out"] for i in range(N_CORES)], axis=0)
    return out.astype(np.float32), res


def kernel(**inputs) -> np.ndarray:
    out, _ = run(inputs, trace=False)
    return out
